# revision 9
# baseline (speedup 1.0000x reference)
"""MoE routing gate kernel for Trainium2 (8 NeuronCores, data-parallel).

Computes, for x[32768, 2048], weight[64, 2048], bias[64]:
    logits = x @ weight.T
    probs  = softmax(logits, axis=-1)
    idx    = top_k(probs + bias, 6).indices
    w      = take_along_axis(probs, idx)
returning (w float32 [32768, 6], idx int32 [32768, 6]).

Sharding: tokens split 4096/core across 8 cores; weight/bias replicated.

Per-core pipeline (memory-bound; HBM floor ~94us for the 33.5MB shard):
  - Host pre-transposes the x shard to xT and splits into bf16 hi/lo
    (x ~= hi + lo); the matmul runs as 3 bf16 passes accumulating fp32
    in PSUM, matching fp32 logits to ~5e-6 while streaming at full PE
    rate.  Host packs xT as [sg, k, p, hi/lo, t] so each DMA moves 2MB
    with 4KB contiguous segments (few, large DMAs: the HWDGE sequencer
    costs ~565ns per dma_start).
  - Matmul pairs are column-tiled: group g=0 lands in PE columns 0-63
    (PSUM partitions 0-63), g=1 in columns 64-127, so two N=512 matmuls
    stream concurrently through disjoint column groups of the array.
  - logits^T [64,512] -> ACT copy to SBUF -> PE transpose (identity
    matmul) back to [128 tokens, 64 experts] tiles in PSUM.
  - Softmax without max-subtraction (|logits| < ~7, exp is safe in
    fp32): ACT exp reads PSUM and emits the row sum via accum_out.
    Selection key q = exp + sum*bias ranks identically to probs + bias.
  - DVE Max8/MaxIndex8 give top-8 values+indices per token; the top-6
    unbiased weights come from 6 fused scalar_tensor_tensor ops:
    (iota == idx_k) * exp with accum_out = the gathered exp, then one
    scale by 1/sum.
"""

import numpy as np
import ml_dtypes

import concourse.bacc as bacc
import concourse.bass as bass
import concourse.mybir as mybir
import concourse.tile as tile
from concourse.bass_utils import run_bass_kernel_spmd

BF16 = mybir.dt.bfloat16
F32 = mybir.dt.float32
I32 = mybir.dt.int32
U32 = mybir.dt.uint32
AX = mybir.AxisListType.X
OP = mybir.AluOpType
EXP = mybir.ActivationFunctionType.Exp

TOKENS, DIM, E, TOPK, NCORES = 32768, 2048, 64, 6, 8
KC = DIM // 128  # contraction chunks of 128
KQ = 4           # k-chunks per DMA


def build_nc(tpc, sg_t=1024):
    """Build the per-core Bass program for a tpc-token shard."""
    grp = sg_t // 2         # tokens per matmul (N), two col-tiled groups per sg
    assert grp == 512
    nsg = tpc // sg_t
    nj = sg_t // 128        # 128-token tiles per super-group
    cols = nj * TOPK        # staging cols per sg

    nc = bacc.Bacc("TRN2", target_bir_lowering=False, debug=False)

    xt = nc.dram_tensor("xt", [nsg, KC, 128, 2, sg_t], BF16, kind="ExternalInput")
    wt_hi = nc.dram_tensor("wt_hi", [128, KC, E], BF16, kind="ExternalInput")
    wt_lo = nc.dram_tensor("wt_lo", [128, KC, E], BF16, kind="ExternalInput")
    bias_b = nc.dram_tensor("bias_b", [128, E], F32, kind="ExternalInput")
    iota64 = nc.dram_tensor("iota64", [128, E], F32, kind="ExternalInput")
    ident2 = nc.dram_tensor("ident2", [128, 64], F32, kind="ExternalInput")
    w_out = nc.dram_tensor("w_out", [nsg, 128, cols], F32, kind="ExternalOutput")
    i_out = nc.dram_tensor("i_out", [nsg, 128, cols], I32, kind="ExternalOutput")

    with tile.TileContext(nc) as tc:
        with (
            tc.tile_pool(name="consts", bufs=1) as cpool,
            tc.tile_pool(name="xbuf", bufs=8) as xpool,
            tc.tile_pool(name="lt", bufs=3) as ltp,
            tc.tile_pool(name="small", bufs=8) as smp,
            tc.tile_pool(name="work", bufs=4) as wkp,
            tc.tile_pool(name="stage", bufs=3) as stp,
            tc.tile_pool(name="acc", bufs=3, space="PSUM") as accp,
            tc.tile_pool(name="tr", bufs=4, space="PSUM") as trp,
        ):
            cwh = cpool.tile([128, KC, E], BF16)
            nc.scalar.dma_start(cwh, wt_hi[:])
            cwl = cpool.tile([128, KC, E], BF16)
            nc.scalar.dma_start(cwl, wt_lo[:])
            cbias = cpool.tile([128, E], F32)
            nc.scalar.dma_start(cbias, bias_b[:])
            ciota = cpool.tile([128, E], F32)
            nc.scalar.dma_start(ciota, iota64[:])
            cident = cpool.tile([128, 64], F32)
            nc.scalar.dma_start(cident, ident2[:])

            for sg in range(nsg):
                # x super-group: KQ-chunk DMAs, [p][k][hi/lo][t] in SBUF
                xq = []
                for kq in range(KC // KQ):
                    xtile = xpool.tile([128, KQ, 2, sg_t], BF16, tag="x")
                    src = xt[sg, kq * KQ:(kq + 1) * KQ].transpose([1, 0, 2, 3])
                    nc.sync.dma_start(xtile, src)
                    xq.append(xtile)

                # 96 matmuls: col-tiled pairs (g=0 -> cols 0-63, g=1 -> 64-127)
                acc = accp.tile([128, grp], F32)
                for k in range(KC):
                    xk = xq[k // KQ][:, k % KQ]  # [128, 2, sg_t]
                    for p in range(3):
                        w = (cwh if p < 2 else cwl)[:, k, :]
                        xs = xk[:, 0 if p != 1 else 1]  # hi, lo, hi
                        first, last = (k == 0 and p == 0), (k == KC - 1 and p == 2)
                        nc.tensor.matmul(
                            acc[0:64], w, xs[:, 0:grp],
                            start=first, stop=last, tile_position=(0, 0),
                        )
                        nc.tensor.matmul(
                            acc[64:128], w, xs[:, grp:sg_t],
                            start=first, stop=last, tile_position=(0, 64),
                            skip_group_check=True,
                        )

                lt = ltp.tile([128, grp], F32)
                nc.scalar.copy(lt, acc)

                sw = stp.tile([128, cols], F32, tag="sw")
                si = stp.tile([128, cols], I32, tag="si")
                for j in range(nj):
                    base = 64 * (j // 4)
                    tps = trp.tile([128, E], F32)
                    nc.tensor.transpose(
                        tps,
                        lt[base:base + 64, (j % 4) * 128:(j % 4 + 1) * 128],
                        cident[base:base + 64, :],
                    )
                    ex = wkp.tile([128, E], F32, tag="ex", bufs=nj + 2)
                    ssum = smp.tile([128, 1], F32, tag="ssum")
                    nc.scalar.activation(ex, tps, EXP, accum_out=ssum)
                    q = wkp.tile([128, E], F32, tag="q")
                    nc.vector.scalar_tensor_tensor(
                        q, cbias, ssum, ex, OP.mult, OP.add
                    )
                    mx = smp.tile([128, 8], F32, tag="mx")
                    nc.vector.max(mx, q)
                    mi = smp.tile([128, 8], U32, tag="mi")
                    nc.vector.max_index(mi, mx, q)
                    idxf = smp.tile([128, 8], F32, tag="idxf")
                    nc.vector.tensor_copy(idxf, mi)
                    rs = smp.tile([128, 1], F32, tag="rs")
                    nc.vector.reciprocal(rs, ssum)
                    col = j * TOPK
                    nc.vector.tensor_copy(si[:, col:col + TOPK], mi[:, 0:TOPK])
                    scr = wkp.tile([128, TOPK, E], F32, tag="scr")
                    g6 = smp.tile([128, TOPK], F32, tag="g6")
                    for kk in range(TOPK):
                        nc.vector.scalar_tensor_tensor(
                            scr[:, kk], ciota, idxf[:, kk:kk + 1], ex,
                            OP.is_equal, OP.mult,
                            accum_out=g6[:, kk:kk + 1],
                        )
                    nc.vector.tensor_scalar_mul(sw[:, col:col + TOPK], g6, rs)
                nc.gpsimd.dma_start(w_out[sg], sw)
                nc.gpsimd.dma_start(i_out[sg], si)
    return nc


_CACHE = {}


def _get_compiled(tpc):
    if tpc not in _CACHE:
        nc = build_nc(tpc)
        nc.compile()
        _CACHE[tpc] = nc
    return _CACHE[tpc]


def _prep_shared(weight, bias):
    bf = ml_dtypes.bfloat16
    w = np.asarray(weight, np.float32)
    w_hi = w.astype(bf)
    w_lo = (w - w_hi.astype(np.float32)).astype(bf)

    def wtile(a):  # [E, DIM] -> [128, KC, E]
        return np.ascontiguousarray(
            np.ascontiguousarray(a.T).reshape(KC, 128, E).transpose(1, 0, 2)
        )

    return {
        "wt_hi": wtile(w_hi),
        "wt_lo": wtile(w_lo),
        "bias_b": np.ascontiguousarray(
            np.broadcast_to(np.asarray(bias, np.float32), (128, E))
        ),
        "iota64": np.ascontiguousarray(
            np.broadcast_to(np.arange(E, dtype=np.float32), (128, E))
        ),
        "ident2": np.ascontiguousarray(
            np.tile(np.eye(64, dtype=np.float32), (2, 1))
        ),
    }


def prep_core_inputs(x, weight, bias, ncores=NCORES, sg_t=1024):
    bf = ml_dtypes.bfloat16
    shared = _prep_shared(weight, bias)
    x = np.asarray(x, np.float32)
    tpc = x.shape[0] // ncores
    nsg = tpc // sg_t
    in_maps = []
    for c in range(ncores):
        xs = np.ascontiguousarray(x[c * tpc:(c + 1) * tpc].T)  # [DIM, tpc]
        xh = xs.astype(bf)
        xl = (xs - xh.astype(np.float32)).astype(bf)
        # pack to [nsg, KC, 128, 2, sg_t]
        pk = np.empty((nsg, KC, 128, 2, sg_t), dtype=bf)
        xh4 = xh.reshape(KC, 128, nsg, sg_t)
        xl4 = xl.reshape(KC, 128, nsg, sg_t)
        pk[:, :, :, 0, :] = xh4.transpose(2, 0, 1, 3)
        pk[:, :, :, 1, :] = xl4.transpose(2, 0, 1, 3)
        in_maps.append({"xt": pk, **shared})
    return in_maps


def unpack_outputs(res_list, tpc):
    ws, idxs = [], []
    for r in res_list:
        wv = np.asarray(r["w_out"])  # [nsg, 128, cols]
        iv = np.asarray(r["i_out"])
        nsg = wv.shape[0]
        wv = wv.reshape(nsg, 128, -1, TOPK).transpose(0, 2, 1, 3).reshape(tpc, TOPK)
        iv = iv.reshape(nsg, 128, -1, TOPK).transpose(0, 2, 1, 3).reshape(tpc, TOPK)
        ws.append(wv)
        idxs.append(iv)
    return (
        np.ascontiguousarray(np.concatenate(ws)).astype(np.float32),
        np.ascontiguousarray(np.concatenate(idxs)).astype(np.int32),
    )


def run(x, weight, bias, trace=False, **kwargs):
    x = np.asarray(x, np.float32)
    tpc = x.shape[0] // NCORES
    nc = _get_compiled(tpc)
    in_maps = prep_core_inputs(x, weight, bias)
    res = run_bass_kernel_spmd(nc, in_maps, list(range(NCORES)), trace=trace, **kwargs)
    w, i = unpack_outputs(res.results, tpc)
    return w, i, res


def kernel(x, weight, bias):
    w, i, _ = run(x, weight, bias, trace=False)
    return w, i


# revision 12
# speedup vs baseline: 1.0287x; 1.0287x over previous
"""MoE routing gate kernel for Trainium2 (8 NeuronCores, data-parallel).

Computes, for x[32768, 2048], weight[64, 2048], bias[64]:
    logits = x @ weight.T
    probs  = softmax(logits, axis=-1)
    idx    = top_k(probs + bias, 6).indices
    w      = take_along_axis(probs, idx)
returning (w float32 [32768, 6], idx int32 [32768, 6]).

Sharding: tokens split 4096/core across 8 cores; weight/bias replicated.

Per-core pipeline (memory-bound; HBM floor ~94us for the 33.5MB shard):
  - Host pre-transposes the x shard to xT and splits into bf16 hi/lo
    (x ~= hi + lo); the matmul runs as 3 bf16 passes accumulating fp32
    in PSUM, matching fp32 logits to ~5e-6 while streaming at full PE
    rate.  Host packs xT as [sg, k, p, hi/lo, t] so each DMA moves 2MB
    with 4KB contiguous segments (few, large DMAs: the HWDGE sequencer
    costs ~565ns per dma_start).
  - Matmul pairs are column-tiled: group g=0 lands in PE columns 0-63
    (PSUM partitions 0-63), g=1 in columns 64-127, so two N=512 matmuls
    stream concurrently through disjoint column groups of the array.
  - logits^T [64,512] -> ACT copy to SBUF -> PE transpose (identity
    matmul) back to [128 tokens, 64 experts] tiles in PSUM.
  - Softmax without max-subtraction (|logits| < ~7, exp is safe in
    fp32): ACT exp reads PSUM and emits the row sum via accum_out.
    Selection key q = exp + sum*bias ranks identically to probs + bias.
  - DVE Max8/MaxIndex8 give top-8 values+indices per token; the top-6
    unbiased weights come from 6 fused scalar_tensor_tensor ops:
    (iota == idx_k) * exp with accum_out = the gathered exp, then one
    scale by 1/sum.
"""

import numpy as np
import ml_dtypes

import concourse.bacc as bacc
import concourse.bass as bass
import concourse.mybir as mybir
import concourse.tile as tile
from concourse.bass_utils import run_bass_kernel_spmd

BF16 = mybir.dt.bfloat16
F32 = mybir.dt.float32
I32 = mybir.dt.int32
U32 = mybir.dt.uint32
AX = mybir.AxisListType.X
OP = mybir.AluOpType
EXP = mybir.ActivationFunctionType.Exp

TOKENS, DIM, E, TOPK, NCORES = 32768, 2048, 64, 6, 8
KC = DIM // 128  # contraction chunks of 128
KQ = 4           # k-chunks per DMA


def build_nc(tpc, sg_t=1024):
    """Build the per-core Bass program for a tpc-token shard."""
    grp = sg_t // 2         # tokens per matmul (N), two col-tiled groups per sg
    assert grp == 512
    nsg = tpc // sg_t
    nj = sg_t // 128        # 128-token tiles per super-group
    cols = nj * TOPK        # staging cols per sg

    nc = bacc.Bacc("TRN2", target_bir_lowering=False, debug=False)

    xt = nc.dram_tensor(
        "xt", [nsg, KC // KQ, 128, KQ, 2, sg_t], BF16, kind="ExternalInput"
    )
    wt_hi = nc.dram_tensor("wt_hi", [128, KC, E], BF16, kind="ExternalInput")
    wt_lo = nc.dram_tensor("wt_lo", [128, KC, E], BF16, kind="ExternalInput")
    bias_b = nc.dram_tensor("bias_b", [128, E], F32, kind="ExternalInput")
    iota64 = nc.dram_tensor("iota64", [128, E], F32, kind="ExternalInput")
    ident2 = nc.dram_tensor("ident2", [128, 64], F32, kind="ExternalInput")
    w_out = nc.dram_tensor("w_out", [nsg, 128, cols], F32, kind="ExternalOutput")
    i_out = nc.dram_tensor("i_out", [nsg, 128, cols], I32, kind="ExternalOutput")

    with tile.TileContext(nc) as tc:
        with (
            tc.tile_pool(name="consts", bufs=1) as cpool,
            tc.tile_pool(name="xbuf", bufs=8) as xpool,
            tc.tile_pool(name="lt", bufs=3) as ltp,
            tc.tile_pool(name="small", bufs=8) as smp,
            tc.tile_pool(name="work", bufs=4) as wkp,
            tc.tile_pool(name="stage", bufs=3) as stp,
            tc.tile_pool(name="acc", bufs=3, space="PSUM") as accp,
            tc.tile_pool(name="tr", bufs=4, space="PSUM") as trp,
        ):
            cwh = cpool.tile([128, KC, E], BF16)
            nc.scalar.dma_start(cwh, wt_hi[:])
            cwl = cpool.tile([128, KC, E], BF16)
            nc.scalar.dma_start(cwl, wt_lo[:])
            cbias = cpool.tile([128, E], F32)
            nc.scalar.dma_start(cbias, bias_b[:])
            ciota = cpool.tile([128, E], F32)
            nc.scalar.dma_start(ciota, iota64[:])
            cident = cpool.tile([128, 64], F32)
            nc.scalar.dma_start(cident, ident2[:])

            for sg in range(nsg):
                # x super-group: KQ-chunk DMAs, [p][k][hi/lo][t] in SBUF
                xq = []
                for kq in range(KC // KQ):
                    xtile = xpool.tile([128, KQ, 2, sg_t], BF16, tag="x")
                    nc.sync.dma_start(xtile, xt[sg, kq])
                    xq.append(xtile)

                # 96 matmuls: col-tiled pairs (g=0 -> cols 0-63, g=1 -> 64-127)
                acc = accp.tile([128, grp], F32)
                for k in range(KC):
                    xk = xq[k // KQ][:, k % KQ]  # [128, 2, sg_t]
                    for p in range(3):
                        w = (cwh if p < 2 else cwl)[:, k, :]
                        xs = xk[:, 0 if p != 1 else 1]  # hi, lo, hi
                        first, last = (k == 0 and p == 0), (k == KC - 1 and p == 2)
                        nc.tensor.matmul(
                            acc[0:64], w, xs[:, 0:grp],
                            start=first, stop=last, tile_position=(0, 0),
                        )
                        nc.tensor.matmul(
                            acc[64:128], w, xs[:, grp:sg_t],
                            start=first, stop=last, tile_position=(0, 64),
                            skip_group_check=True,
                        )

                lt = ltp.tile([128, grp], F32)
                nc.scalar.copy(lt, acc)

                sw = stp.tile([128, cols], F32, tag="sw")
                si = stp.tile([128, cols], I32, tag="si")
                for j in range(nj):
                    base = 64 * (j // 4)
                    tps = trp.tile([128, E], F32)
                    nc.tensor.transpose(
                        tps,
                        lt[base:base + 64, (j % 4) * 128:(j % 4 + 1) * 128],
                        cident[base:base + 64, :],
                    )
                    ex = wkp.tile([128, E], F32, tag="ex", bufs=nj + 2)
                    ssum = smp.tile([128, 1], F32, tag="ssum")
                    nc.scalar.activation(ex, tps, EXP, accum_out=ssum)
                    q = wkp.tile([128, E], F32, tag="q")
                    nc.vector.scalar_tensor_tensor(
                        q, cbias, ssum, ex, OP.mult, OP.add
                    )
                    mx = smp.tile([128, 8], F32, tag="mx")
                    nc.vector.max(mx, q)
                    mi = smp.tile([128, 8], U32, tag="mi")
                    nc.vector.max_index(mi, mx, q)
                    idxf = smp.tile([128, 8], F32, tag="idxf")
                    nc.vector.tensor_copy(idxf, mi)
                    rs = smp.tile([128, 1], F32, tag="rs")
                    nc.vector.reciprocal(rs, ssum)
                    col = j * TOPK
                    nc.vector.tensor_copy(si[:, col:col + TOPK], mi[:, 0:TOPK])
                    scr = wkp.tile([128, TOPK, E], F32, tag="scr")
                    g6 = smp.tile([128, TOPK], F32, tag="g6")
                    for kk in range(TOPK):
                        nc.vector.scalar_tensor_tensor(
                            scr[:, kk], ciota, idxf[:, kk:kk + 1], ex,
                            OP.is_equal, OP.mult,
                            accum_out=g6[:, kk:kk + 1],
                        )
                    nc.vector.tensor_scalar_mul(sw[:, col:col + TOPK], g6, rs)
                nc.gpsimd.dma_start(w_out[sg], sw)
                nc.gpsimd.dma_start(i_out[sg], si)
    return nc


_CACHE = {}


def _get_compiled(tpc):
    if tpc not in _CACHE:
        nc = build_nc(tpc)
        nc.compile()
        _CACHE[tpc] = nc
    return _CACHE[tpc]


def _prep_shared(weight, bias):
    bf = ml_dtypes.bfloat16
    w = np.asarray(weight, np.float32)
    w_hi = w.astype(bf)
    w_lo = (w - w_hi.astype(np.float32)).astype(bf)

    def wtile(a):  # [E, DIM] -> [128, KC, E]
        return np.ascontiguousarray(
            np.ascontiguousarray(a.T).reshape(KC, 128, E).transpose(1, 0, 2)
        )

    return {
        "wt_hi": wtile(w_hi),
        "wt_lo": wtile(w_lo),
        "bias_b": np.ascontiguousarray(
            np.broadcast_to(np.asarray(bias, np.float32), (128, E))
        ),
        "iota64": np.ascontiguousarray(
            np.broadcast_to(np.arange(E, dtype=np.float32), (128, E))
        ),
        "ident2": np.ascontiguousarray(
            np.tile(np.eye(64, dtype=np.float32), (2, 1))
        ),
    }


def prep_core_inputs(x, weight, bias, ncores=NCORES, sg_t=1024):
    bf = ml_dtypes.bfloat16
    shared = _prep_shared(weight, bias)
    x = np.asarray(x, np.float32)
    tpc = x.shape[0] // ncores
    nsg = tpc // sg_t
    in_maps = []
    for c in range(ncores):
        xs = np.ascontiguousarray(x[c * tpc:(c + 1) * tpc].T)  # [DIM, tpc]
        xh = xs.astype(bf)
        xl = (xs - xh.astype(np.float32)).astype(bf)
        # pack to [nsg, KC//KQ, 128, KQ, 2, sg_t]: per (sg, kq, partition)
        # the [KQ, 2, sg_t] block is one 16KB contiguous run in DRAM
        pk = np.empty((nsg, KC // KQ, 128, KQ, 2, sg_t), dtype=bf)
        xh6 = xh.reshape(KC // KQ, KQ, 128, nsg, sg_t)
        xl6 = xl.reshape(KC // KQ, KQ, 128, nsg, sg_t)
        pk[:, :, :, :, 0, :] = xh6.transpose(3, 0, 2, 1, 4)
        pk[:, :, :, :, 1, :] = xl6.transpose(3, 0, 2, 1, 4)
        in_maps.append({"xt": pk, **shared})
    return in_maps


def unpack_outputs(res_list, tpc):
    ws, idxs = [], []
    for r in res_list:
        wv = np.asarray(r["w_out"])  # [nsg, 128, cols]
        iv = np.asarray(r["i_out"])
        nsg = wv.shape[0]
        wv = wv.reshape(nsg, 128, -1, TOPK).transpose(0, 2, 1, 3).reshape(tpc, TOPK)
        iv = iv.reshape(nsg, 128, -1, TOPK).transpose(0, 2, 1, 3).reshape(tpc, TOPK)
        ws.append(wv)
        idxs.append(iv)
    return (
        np.ascontiguousarray(np.concatenate(ws)).astype(np.float32),
        np.ascontiguousarray(np.concatenate(idxs)).astype(np.int32),
    )


def run(x, weight, bias, trace=False, **kwargs):
    x = np.asarray(x, np.float32)
    tpc = x.shape[0] // NCORES
    nc = _get_compiled(tpc)
    in_maps = prep_core_inputs(x, weight, bias)
    res = run_bass_kernel_spmd(nc, in_maps, list(range(NCORES)), trace=trace, **kwargs)
    w, i = unpack_outputs(res.results, tpc)
    return w, i, res


def kernel(x, weight, bias):
    w, i, _ = run(x, weight, bias, trace=False)
    return w, i
